# revision 16
# baseline (speedup 1.0000x reference)
"""Trainium2 Bass kernel for nn_CharRNN: 2-layer MI-GRU + large vocab projection.

Strategy (8 NeuronCores, SPMD, no collectives):
  - BATCH-SHARDED recurrence: B=100 padded to 104 = 8 x 13. Each core runs
    the full T=50 recurrence for its 13 batch rows in "T-layout": hidden
    state kept transposed as hT [128 partitions, 4 chunks, 13 batch], all
    matmuls out[feature-tile, batch] = U_tile.T @ hT with the weight tile
    stationary and the tiny batch dim moving.
  - Recurrence matmuls at FD=13 are LDWEIGHTS-bound (~64cyc fp16 FWL per
    128x128 tile).  The schedule therefore minimizes weight-tile streams
    per step and fills PE wait gaps with ready work:
      * cell1 (layer 1) is LAGGED by 5 steps behind cell0.  The W1 input
        path Wx1 = W1f.T @ h0 is batched per 4-step half-window at FD=52,
        nearly amortizing its LDWEIGHTS (48 tiles per 4 steps instead of
        48 tiles per step).
      * projection work is cut into small chunks (4 vocab tiles = 16 MMs)
        and emitted as fillers at several points INSIDE each step, so the
        in-order PE always has ready work during the cell's vector/act
        chain waits.
  - Critical chain tightening: cell0's r-gate path is split from the
    z-gate path (r: 16 MMs -> STT_r -> SIG_r -> rh -> c MMs), the h-update
    tail is 2 ops (tmp = (z-1)*c_neg; h_new = tmp + zh) instead of 3, and
    cell0 runs on DVE while cell1 runs on Pool so the two lagged chains
    don't queue behind each other.
  - Layer-0's input path A0 = alpha*wx0+beta1 is precomputed on the host
    for all timesteps and streamed to SBUF.
  - Projection: logitsT[vocab-tile, rows] = wsm_tile.T @ h1T over 8-step
    windows (rows=104); host undoes the transpose when assembling.

Gate algebra (MI-GRU), folded on host:
    gate_arg = alpha*wx*uh + beta1*uh + beta2*wx + b
             = (uh + beta2/alpha) * (alpha*wx + beta1) + (b - beta1*beta2/alpha)
    A := alpha*wx + beta1 (host for layer 0; window-batched evac for layer 1)
    m := (ps_uh + r2) * A            (one STT per gate group)
    r,z = sigmoid(m + d_g)
    c_neg = tanh(-(m_c + d_c)) = -c
    h_new = zh + (z-1)*c_neg  with zh = z*h, zm1 = z-1 precomputed off-path
"""

import os
import sys

sys.path.insert(0, "/opt/trn_rl_repo")

import numpy as np

import concourse.bass as bass
import concourse.mybir as mybir
import concourse.tile as tile

# ----------------------------------------------------------------------------
# Patch: the final SP Drain emitted by TileContext collects one semaphore wait
# per busy logical processor, but the walrus build in this container only
# lowers a limited number of sync-wait commands per CTRL instruction.  Split
# the waits across separate single-wait NoOps.
# ----------------------------------------------------------------------------
from concourse.vector_clock import ScopedClock
from bass_rust import SyncInfo

_MAXW = 1


def _patched_drain_and_barrier(self, tick_clock, wait_clock):
    nc = self.nc
    drain_inst = nc.sync.drain()
    wait_clock.add_sem_waits(
        drain_inst.ins, ScopedClock({None: tick_clock.global_clock})
    )
    si = drain_inst.ins.sync_info
    waits = list(si.on_wait) if si is not None else []
    if len(waits) > _MAXW:
        drain_inst.ins.sync_info = SyncInfo(
            on_wait=waits[:_MAXW], on_update=list(si.on_update)
        )
        for k in range(_MAXW, len(waits), _MAXW):
            nop = nc.sync.nop(nofuse=True)
            nop.ins.sync_info = SyncInfo(on_wait=waits[k : k + _MAXW], on_update=[])

    nc.all_engine_barrier()
    assert self.sems is not None
    popped = nc._tile_sem_poison_stack.pop()
    assert popped is self._sem_poison
    nc.clear_and_free_semaphores(list(self.sems.allocated().values()))
    nc.all_engine_barrier()


tile.TileContext._drain_and_barrier = _patched_drain_and_barrier

# ----------------------------------------------------------------------------
# Same walrus limitation applies to every engine instruction: split any
# instruction carrying more than _JLIM semaphore waits into preceding
# single-wait NoOps on the same engine (engines are in-order, so blocking on
# a prior NoOp is equivalent).  Done as a BIR-JSON post-pass on serialization.
# ----------------------------------------------------------------------------
import json as _json

_JLIM = 1
_orig_to_json_bytes = bass.Bass.to_json_bytes


def _split_waits_json(self) -> bytes:
    raw = _orig_to_json_bytes(self)
    d = _json.loads(raw)
    ctr = [0]

    def fix_block(blk):
        insts = blk.get("instructions")
        if insts:
            out = []
            for ins in insts:
                si = ins.get("sync_info")
                waits = (si or {}).get("on_wait") or []
                if len(waits) > _JLIM:
                    keep = waits[:_JLIM]
                    extra = waits[_JLIM:]
                    for k in range(0, len(extra), _JLIM):
                        ctr[0] += 1
                        out.append(
                            {
                                "debug": ins.get("debug", 0),
                                "engine": ins["engine"],
                                "ins": [],
                                "name": f"I-sw{ctr[0]}",
                                "opcode": "NoOp",
                                "outs": [],
                                "sync_info": {
                                    "on_wait": extra[k : k + _JLIM],
                                    "on_update": [],
                                },
                            }
                        )
                    si["on_wait"] = keep
                out.append(ins)
            blk["instructions"] = out
        for sub in blk.get("blocks", []) or []:
            fix_block(sub)

    for f in d.get("functions", []):
        for blk in f.get("blocks", []) or []:
            fix_block(blk)
    return _json.dumps(d).encode()


bass.Bass.to_json_bytes = _split_waits_json

# ----------------------------------------------------------------------------

B, T, H, E, V = 100, 50, 512, 128, 8000
G = 3 * H  # 1536
NCORES = 8
BP = 104  # padded batch
BC = BP // NCORES  # 13 batch rows per core
KH = H // 128  # 4 K-chunks for H contraction
VPAD = 8064  # vocab padded to 63*128
NVT = VPAD // 128  # 63 vocab tiles

HWIN = 4  # half-window for W1 batching (steps)
LAG = 5  # cell1 lag in steps (>= HWIN + 1)
WIN = 8  # projection window (steps)
ROWS_W = WIN * BC  # 104 projection rows per full window
VROUND = 4  # vocab tiles per projection chunk
NCHUNK = (NVT + VROUND - 1) // VROUND  # 16 chunks per window

FP16 = mybir.dt.float16
F32 = mybir.dt.float32
AF = mybir.ActivationFunctionType
ALU = mybir.AluOpType

LAST_RESULTS = None


def _const_scalar(row, name):
    row = np.asarray(row, dtype=np.float64)
    lo, hi = row.min(), row.max()
    assert hi - lo < 1e-12, f"{name} is not a constant row; fast path invalid"
    return float(row[0])


def _fp16(a):
    return np.ascontiguousarray(np.asarray(a, dtype=np.float32)).astype(np.float16)


def _fold_layer(W, U, b, alpha, beta1, beta2):
    """Host folding: Wf = W*alpha; per-range scalars for the gate algebra."""
    W, U = np.asarray(W, np.float64), np.asarray(U, np.float64)
    alpha = np.asarray(alpha, np.float64)
    beta1 = np.asarray(beta1, np.float64)
    beta2 = np.asarray(beta2, np.float64)
    b = np.asarray(b, np.float64)
    Wf = W * alpha[None, :]
    r2 = beta2 / alpha
    d = b - beta1 * beta2 / alpha
    sc = {
        "b1g": _const_scalar(beta1[: 2 * H], "beta1_g"),
        "b1c": _const_scalar(beta1[2 * H :], "beta1_c"),
        "r2g": _const_scalar(r2[: 2 * H], "r2_g"),
        "r2c": _const_scalar(r2[2 * H :], "r2_c"),
        "dg": _const_scalar(d[: 2 * H], "d_g"),
        "dc": _const_scalar(d[2 * H :], "d_c"),
    }
    return Wf, np.asarray(U, np.float64), sc


def _tiles_T(M):
    """[K, N] weight matrix -> [128, nk*nt*128] fp16 laid out (p, k, tile, col)
    so sbuf[:, k, j, :] is the stationary lhsT tile for chunk k, out-tile j."""
    K, N = M.shape
    nk, nt = K // 128, N // 128
    t = np.asarray(M, np.float32).reshape(nk, 128, nt, 128).transpose(1, 0, 2, 3)
    return _fp16(t.reshape(128, nk * nt * 128))


def _build_program(sc0, sc1, zero_bias):
    nc = bass.Bass(
        "TRN2", target_bir_lowering=False, debug=False, num_devices=NCORES
    )

    u0rz_d = nc.dram_tensor("u0rz", [128, KH, 8, 128], FP16, kind="ExternalInput").ap()
    u0c_d = nc.dram_tensor("u0c", [128, KH, 4, 128], FP16, kind="ExternalInput").ap()
    w1_d = nc.dram_tensor("w1", [128, KH, 12, 128], FP16, kind="ExternalInput").ap()
    u1rz_d = nc.dram_tensor("u1rz", [128, KH, 8, 128], FP16, kind="ExternalInput").ap()
    u1c_d = nc.dram_tensor("u1c", [128, KH, 4, 128], FP16, kind="ExternalInput").ap()
    wsm_d = nc.dram_tensor("wsm", [128, KH, NVT, 128], FP16, kind="ExternalInput").ap()
    a0_d = nc.dram_tensor("a0", [128, T, 12, BC], F32, kind="ExternalInput").ap()
    sbT_d = nc.dram_tensor("sbT", [128, NVT], F32, kind="ExternalInput").ap()
    out_d = nc.dram_tensor("out", [NVT, 128, T * BC], FP16, kind="ExternalOutput").ap()

    def build(tc):
        nc = tc.nc
        cpool = tc.alloc_tile_pool(name="const", bufs=1)

        u0rz_s = cpool.tile([128, KH, 8, 128], FP16, tag="u0rz")
        u0c_s = cpool.tile([128, KH, 4, 128], FP16, tag="u0c")
        w1_s = cpool.tile([128, KH, 12, 128], FP16, tag="w1")
        u1rz_s = cpool.tile([128, KH, 8, 128], FP16, tag="u1rz")
        u1c_s = cpool.tile([128, KH, 4, 128], FP16, tag="u1c")
        wsm_s = cpool.tile([128, KH, NVT, 128], FP16, tag="wsm")
        a0a_s = cpool.tile([128, 10, 12, BC], F32, tag="a0a")
        a0b_s = cpool.tile([128, T - 10, 12, BC], F32, tag="a0b")
        sbT_s = cpool.tile([128, NVT], F32, tag="sbT")

        warm = cpool.tile([1, 2], F32, tag="warm")
        nc.vector.memset(warm[:], 0.0)
        nc.scalar.activation(warm[:], warm[:], AF.Sigmoid, bias=0.0)
        # first-needed loads up front; the rest streamed inside the loop
        nc.sync.dma_start(u0rz_s[:, :, :, :], u0rz_d[:, :, :, :])
        nc.scalar.dma_start(a0a_s[:, :, :, :], a0_d[:, 0:10, :, :])
        nc.gpsimd.dma_start(u0c_s[:, :, :, :], u0c_d[:, :, :, :])

        _bias_tiles = {}

        def bias_ap(val, parts=128):
            val = float(val)
            if val not in _bias_tiles:
                bt = cpool.tile([128, 1], F32, tag=f"bias_{len(_bias_tiles)}")
                nc.vector.memset(bt[:], val)
                _bias_tiles[val] = bt
            return _bias_tiles[val][:parts]

        # pools
        ps = tc.alloc_tile_pool(name="ps", bufs=1, space="PSUM")
        pswx = tc.alloc_tile_pool(name="pswx", bufs=1, space="PSUM")
        psp = tc.alloc_tile_pool(name="psp", bufs=3, space="PSUM")
        sb = tc.alloc_tile_pool(name="sb", bufs=2)
        h0win = tc.alloc_tile_pool(name="h0win", bufs=3)
        h1win = tc.alloc_tile_pool(name="h1win", bufs=3)
        a1win = tc.alloc_tile_pool(name="a1win", bufs=2)

        # initial states (zeros)
        h0T_init = cpool.tile([128, KH, BC], FP16, tag="h0init")
        nc.vector.memset(h0T_init[:], 0.0)
        h1T_init = cpool.tile([128, KH, BC], FP16, tag="h1init")
        nc.gpsimd.memset(h1T_init[:], 0.0)

        # ------------------------------------------------------------------
        # filler machinery: small callables emitting ready PE work, popped
        # at insertion points inside the cells so the in-order PE stays fed
        # ------------------------------------------------------------------
        fillers = []  # entries: (added_tick, emit_fn)

        def pop_filler(n=1):
            for _ in range(n):
                if fillers:
                    fillers.pop(0)[1]()

        def flush_stale(t):
            # every filler must be emitted at most one tick after queueing so
            # the lagged cell1 never reads an A1 window that hasn't been
            # emitted yet (emission order defines the dependency graph)
            while fillers and fillers[0][0] < t:
                fillers.pop(0)[1]()

        # ------------------------------------------------------------------
        def cell0(t, hT, hslot, A_t, otile, oslot):
            """Layer-0 cell, FD=13, critical chain on DVE with r/z split."""

            def hk(k):
                return hT[:, k, :] if hslot is None else hT[:, k, hslot, :]

            h_full = hT[:, :, :] if hslot is None else hT[:, :, hslot, :]
            ps_cell = ps.tile([128, 12, BC], F32, tag="pscell0")
            # r MMs (j=0..3), then z MMs (j=4..7)
            for j in range(8):
                for k in range(KH):
                    nc.tensor.matmul(
                        ps_cell[:, j, :],
                        u0rz_s[:, k, j, :],
                        hk(k),
                        start=(k == 0),
                        stop=(k == KH - 1),
                    )
                if j == 3:
                    pop_filler()
            m_r = sb.tile([128, 4, BC], F32, tag="mr0")
            nc.vector.scalar_tensor_tensor(
                m_r[:], ps_cell[:, 0:4, :], sc0["r2g"], A_t[:, 0:4, :],
                ALU.add, ALU.mult,
            )
            r_g = sb.tile([128, 4, BC], FP16, tag="r0")
            nc.scalar.activation(r_g[:], m_r[:], AF.Sigmoid, bias=bias_ap(sc0["dg"]))
            rh = sb.tile([128, KH, BC], FP16, tag="rh0")
            nc.vector.tensor_tensor(rh[:], r_g[:], h_full, ALU.mult)
            for j in range(4):
                for k in range(KH):
                    nc.tensor.matmul(
                        ps_cell[:, 8 + j, :],
                        u0c_s[:, k, j, :],
                        rh[:, k, :],
                        start=(k == 0),
                        stop=(k == KH - 1),
                    )
            pop_filler()
            m_z = sb.tile([128, 4, BC], F32, tag="mz0")
            nc.vector.scalar_tensor_tensor(
                m_z[:], ps_cell[:, 4:8, :], sc0["r2g"], A_t[:, 4:8, :],
                ALU.add, ALU.mult,
            )
            z_g = sb.tile([128, 4, BC], FP16, tag="z0")
            nc.scalar.activation(z_g[:], m_z[:], AF.Sigmoid, bias=bias_ap(sc0["dg"]))
            zh = sb.tile([128, KH, BC], FP16, tag="zh0")
            nc.vector.tensor_tensor(zh[:], z_g[:], h_full, ALU.mult)
            zm1 = sb.tile([128, 4, BC], FP16, tag="zm10")
            nc.vector.tensor_scalar_sub(zm1[:], z_g[:], 1.0)
            m_c = sb.tile([128, 4, BC], F32, tag="mc0")
            nc.vector.scalar_tensor_tensor(
                m_c[:], ps_cell[:, 8:12, :], sc0["r2c"], A_t[:, 8:12, :],
                ALU.add, ALU.mult,
            )
            c_neg = sb.tile([128, 4, BC], FP16, tag="cn0")
            nc.scalar.activation(
                c_neg[:], m_c[:], AF.Tanh, bias=bias_ap(-sc0["dc"]), scale=-1.0
            )
            tmp = sb.tile([128, 4, BC], FP16, tag="tmp0")
            nc.vector.tensor_tensor(tmp[:], zm1[:], c_neg[:], ALU.mult)
            oap = otile[:, :, :] if oslot is None else otile[:, :, oslot, :]
            nc.vector.tensor_tensor(oap, tmp[:], zh[:], ALU.add)
            pop_filler()

        # ------------------------------------------------------------------
        def cell1(s, hT, hslot, A_t, otile, oslot):
            """Layer-1 cell, lagged; merged rz, vector work on Pool."""

            def hk(k):
                return hT[:, k, :] if hslot is None else hT[:, k, hslot, :]

            h_full = hT[:, :, :] if hslot is None else hT[:, :, hslot, :]
            ps_cell = ps.tile([128, 12, BC], F32, tag="pscell1")
            for j in range(8):
                for k in range(KH):
                    nc.tensor.matmul(
                        ps_cell[:, j, :],
                        u1rz_s[:, k, j, :],
                        hk(k),
                        start=(k == 0),
                        stop=(k == KH - 1),
                    )
                if j == 3:
                    pop_filler()
            m_rz = sb.tile([128, 8, BC], F32, tag="mrz1")
            nc.vector.scalar_tensor_tensor(
                m_rz[:], ps_cell[:, 0:8, :], sc1["r2g"], A_t[:, 0:8, :],
                ALU.add, ALU.mult,
            )
            rz = sb.tile([128, 8, BC], FP16, tag="rz1")
            nc.scalar.activation(rz[:], m_rz[:], AF.Sigmoid, bias=bias_ap(sc1["dg"]))
            rh = sb.tile([128, KH, BC], FP16, tag="rh1")
            nc.gpsimd.tensor_tensor(rh[:], rz[:, 0:4, :], h_full, ALU.mult)
            for j in range(4):
                for k in range(KH):
                    nc.tensor.matmul(
                        ps_cell[:, 8 + j, :],
                        u1c_s[:, k, j, :],
                        rh[:, k, :],
                        start=(k == 0),
                        stop=(k == KH - 1),
                    )
            pop_filler()
            zh = sb.tile([128, KH, BC], FP16, tag="zh1")
            nc.gpsimd.tensor_tensor(zh[:], rz[:, 4:8, :], h_full, ALU.mult)
            zm1 = sb.tile([128, 4, BC], FP16, tag="zm11")
            nc.gpsimd.tensor_scalar_sub(zm1[:], rz[:, 4:8, :], 1.0)
            m_c = sb.tile([128, 4, BC], F32, tag="mc1")
            nc.vector.scalar_tensor_tensor(
                m_c[:], ps_cell[:, 8:12, :], sc1["r2c"], A_t[:, 8:12, :],
                ALU.add, ALU.mult,
            )
            c_neg = sb.tile([128, 4, BC], FP16, tag="cn1")
            nc.scalar.activation(
                c_neg[:], m_c[:], AF.Tanh, bias=bias_ap(-sc1["dc"]), scale=-1.0
            )
            tmp = sb.tile([128, 4, BC], FP16, tag="tmp1")
            nc.gpsimd.tensor_tensor(tmp[:], zm1[:], c_neg[:], ALU.mult)
            oap = otile[:, :, :] if oslot is None else otile[:, :, oslot, :]
            nc.gpsimd.tensor_tensor(oap, tmp[:], zh[:], ALU.add)
            pop_filler()

        # ------------------------------------------------------------------
        proj_ctr = [0]

        def make_proj_chunk(w, ci, hist, nsteps, off=0, dma_eng=None):
            """Projection chunk: vocab tiles [ci*VROUND, ...), FD=nsteps*BC."""

            def emit():
                vt0 = ci * VROUND
                nvt = min(VROUND, NVT - vt0)
                nrows = nsteps * BC
                pt = psp.tile([128, VROUND, 128], F32, tag="pproj")
                for j in range(nvt):
                    vt = vt0 + j
                    for k in range(KH):
                        nc.tensor.matmul(
                            pt[:, j, :nrows],
                            wsm_s[:, k, vt, :],
                            hist[:, k, off : off + nsteps, :],
                            start=(k == 0),
                            stop=(k == KH - 1),
                        )
                lo = sb.tile([128, VROUND, ROWS_W], FP16, tag="lproj", bufs=4)
                idx = proj_ctr[0]
                proj_ctr[0] += 1
                if zero_bias:
                    if idx % 2 == 0:
                        nc.vector.tensor_copy(lo[:, :nvt, :nrows], pt[:, :nvt, :nrows])
                    else:
                        nc.scalar.copy(lo[:, :nvt, :nrows], pt[:, :nvt, :nrows])
                else:
                    for j in range(nvt):
                        vt = vt0 + j
                        if idx % 2 == 0:
                            nc.vector.tensor_scalar(
                                lo[:, j, :nrows], pt[:, j, :nrows],
                                sbT_s[:, vt : vt + 1], None, ALU.add,
                            )
                        else:
                            nc.scalar.activation(
                                lo[:, j, :nrows], pt[:, j, :nrows],
                                AF.Identity, bias=sbT_s[:, vt : vt + 1],
                            )
                dst = out_d[
                    vt0 : vt0 + nvt, :, w * ROWS_W : w * ROWS_W + nrows
                ].rearrange("j p r -> p j r")
                nc.sync.dma_start(dst, lo[:, :nvt, :nrows])

            return emit

        def make_w1_group(g, hist, nsteps, wx_tile):
            """W1 window-batched MMs for j-tiles [3g, 3g+3), FD=nsteps*BC."""

            def emit():
                nrows = nsteps * BC
                for j in range(3 * g, 3 * g + 3):
                    for k in range(KH):
                        nc.tensor.matmul(
                            wx_tile[:, j, :nrows],
                            w1_s[:, k, j, :],
                            hist[:, k, :nsteps, :],
                            start=(k == 0),
                            stop=(k == KH - 1),
                        )

            return emit

        def make_a1_evac(p, nsteps, wx_tile, a1_tile):
            """Evac slots [2p, 2p+2) of the Wx window: A1 = ps + b1g (Pool)."""

            def emit():
                lo = 2 * p * BC
                hi = min((2 * p + 2) * BC, nsteps * BC)
                nc.vector.tensor_scalar_add(
                    a1_tile[:, :, lo:hi], wx_tile[:, :, lo:hi], sc1["b1g"]
                )

            return emit

        def make_dummy(j0):
            def emit():
                pt = psp.tile([128, VROUND, 128], F32, tag="pproj")
                for j in range(4):
                    for k in range(KH):
                        nc.tensor.matmul(
                            pt[:, j, :26],
                            u0rz_s[:, k, (j0 + j) % 8, :],
                            a0_s[:, 0:2, j, :],
                            start=(k == 0),
                            stop=(k == KH - 1),
                        )
            return emit

        # ---- main loop ----
        NHW = (T + HWIN - 1) // HWIN  # 13 half-windows (12x4 + 1x2)
        h0_tile, h0_slot = h0T_init, None
        h1_tile, h1_slot = h1T_init, None
        h0hw = None  # current half-window h0 history
        h0hw_list = [None] * NHW
        a1_list = [None] * NHW
        h1hist = None
        h1hist_list = [None] * (T // WIN + 1)
        proj_jobs = []
        total_ticks = T + LAG

        for t in range(total_ticks):
            # stream big loads in pieces early on
            if t < T:
                if 2 <= t <= 5:
                    i = t - 2
                    nc.gpsimd.dma_start(
                        a0b_s[:, i * 10 : i * 10 + 10, :, :],
                        a0_d[:, 10 + i * 10 : 20 + i * 10, :, :],
                    )
                if 2 <= t <= 9:
                    for sub in range(4):
                        k, piece = (t - 2) // 2, (t - 2) % 2 * 4 + sub
                        eng = nc.scalar if sub % 2 == 0 else nc.sync
                        vlo = piece * 8
                        vhi = min(vlo + 8, NVT)
                        if vlo < NVT:
                            eng.dma_start(
                                wsm_s[:, k, vlo:vhi, :], wsm_d[:, k, vlo:vhi, :]
                            )
                if t == 7 and not zero_bias:
                    nc.scalar.dma_start(sbT_s[:, :], sbT_d[:, :])
                if t == 0:
                    for k in range(KH):
                        nc.scalar.dma_start(u1rz_s[:, k, :, :], u1rz_d[:, k, :, :])
                    nc.scalar.dma_start(u1c_s[:, 0:2, :, :], u1c_d[:, 0:2, :, :])
                    nc.scalar.dma_start(u1c_s[:, 2:4, :, :], u1c_d[:, 2:4, :, :])
                    nc.scalar.dma_start(w1_s[:, :, 0:6, :], w1_d[:, :, 0:6, :])
                    nc.scalar.dma_start(w1_s[:, :, 6:12, :], w1_d[:, :, 6:12, :])

            # pop a couple of extra proj chunks per tick beyond the in-cell
            # insertion points when the queue is deep
            if len(fillers) > 8:
                pop_filler(1)

            # ---- cell0 at step t ----
            if t < T:
                hw = t // HWIN
                hslot_new = t % HWIN
                hwn = min(HWIN, T - hw * HWIN)
                if hslot_new == 0:
                    h0hw = h0win.tile([128, KH, HWIN, BC], FP16, tag="h0hw")
                    h0hw_list[hw] = h0hw
                cell0(
                    t,
                    h0_tile,
                    h0_slot,
                    (a0a_s if t < 10 else a0b_s)[:, t if t < 10 else t - 10, :, :],
                    h0hw,
                    hslot_new,
                )
                h0_tile, h0_slot = h0hw, hslot_new

                # end of half-window: queue W1win groups + A1 evacs
                if hslot_new == hwn - 1:
                    wx_tile = pswx.tile([128, 12, 64], F32, tag="wxwin")
                    a1_tile = a1win.tile([128, 12, HWIN * BC], F32, tag="a1win")
                    a1_list[hw] = a1_tile
                    for g in range(4):
                        fillers.append(
                            (t, make_w1_group(g, h0hw_list[hw], hwn, wx_tile))
                        )
                    for p in range((hwn + 1) // 2):
                        fillers.append((t, make_a1_evac(p, hwn, wx_tile, a1_tile)))

            # ---- cell1 at step s = t - LAG ----
            s = t - LAG
            if 0 <= s < T:
                w = s // WIN
                slot = s % WIN
                if slot == 0:
                    h1hist = h1win.tile([128, KH, WIN, BC], FP16, tag="h1hist")
                    h1hist_list[w] = h1hist
                shw = s // HWIN
                sslot = s % HWIN
                a1_t = a1_list[shw][:, :, sslot * BC : (sslot + 1) * BC]
                cell1(s, h1_tile, h1_slot, a1_t, h1hist, slot)
                h1_tile, h1_slot = h1hist, slot

                # window complete -> enqueue proj chunks
                if slot == WIN - 1 or s == T - 1:
                    nst = slot + 1
                    for ci in range(NCHUNK):
                        proj_jobs.append((w, ci, h1hist, nst))

            # feed proj jobs into the filler queue (rate-matched)
            navail = 3 if t >= 13 else 2
            while navail > 0 and proj_jobs:
                fillers.append((t, make_proj_chunk(*proj_jobs.pop(0))))
                navail -= 1

            flush_stale(t)

        # drain remaining proj work
        di = 0
        while proj_jobs:
            eng = nc.gpsimd if di % 4 == 1 else nc.sync
            w_, ci_, h_, n_, o_ = proj_jobs.pop(0)
            fillers.append(
                (total_ticks, make_proj_chunk(w_, ci_, h_, n_, o_, dma_eng=eng))
            )
            di += 1
        pop_filler(len(fillers))

        for p in (a1win, h1win, h0win, sb, psp, pswx, ps, cpool):
            p.release()

    return nc, build


def build_and_prep(inputs):
    """Build the Bass program and per-core input maps. Shared by kernel()
    and offline sim benches."""
    inp = {k: np.asarray(v) for k, v in inputs.items()}

    W0f, U0, sc0 = _fold_layer(
        inp["W0"], inp["U0"], inp["b0"], inp["alpha0"], inp["beta1_0"], inp["beta2_0"]
    )
    W1f, U1, sc1 = _fold_layer(
        inp["W1"], inp["U1"], inp["b1"], inp["alpha1"], inp["beta1_1"], inp["beta2_1"]
    )
    for sc in (sc0, sc1):
        assert abs(sc["b1g"] - sc["b1c"]) < 1e-12, "A-move needs split biases"

    # host: embedding lookup + full layer-0 input path
    idx = np.asarray(inp["input_data"]).astype(np.int64)  # [B, T]
    idx_p = np.concatenate([idx, np.zeros((BP - B, T), np.int64)], axis=0)
    xs = np.asarray(inp["embedding"], np.float64)[idx_p]  # [BP, T, E]
    A0 = xs @ W0f + sc0["b1g"]  # [BP, T, G]
    A0t = A0.reshape(BP, T, 12, 128).transpose(3, 1, 2, 0)  # [128, T, 12, BP]
    A0t = np.ascontiguousarray(A0t, np.float32)

    wsm = np.zeros((H, VPAD), np.float32)
    wsm[:, :V] = np.asarray(inp["softmax_w"], np.float32)
    sbv = np.zeros((VPAD,), np.float32)
    sbv[:V] = np.asarray(inp["softmax_b"], np.float32)

    base_map = {
        "u0rz": _tiles_T(U0[:, : 2 * H]).reshape(128, KH, 8, 128),
        "u0c": _tiles_T(U0[:, 2 * H :]).reshape(128, KH, 4, 128),
        "w1": _tiles_T(W1f).reshape(128, KH, 12, 128),
        "u1rz": _tiles_T(U1[:, : 2 * H]).reshape(128, KH, 8, 128),
        "u1c": _tiles_T(U1[:, 2 * H :]).reshape(128, KH, 4, 128),
        "wsm": _tiles_T(wsm).reshape(128, KH, NVT, 128),
        "sbT": np.ascontiguousarray(sbv.reshape(NVT, 128).T, np.float32),
    }

    nc, build = _build_program(sc0, sc1, zero_bias=bool(np.all(sbv == 0)))
    with tile.TileContext(nc) as tc:
        build(tc)

    in_maps = []
    for c in range(NCORES):
        m = dict(base_map)
        m["a0"] = np.ascontiguousarray(A0t[:, :, :, c * BC : (c + 1) * BC])
        in_maps.append(m)

    def assemble(results):
        # results[c]["out"]: [VPAD, T*BC], col = t*BC + j  (b = c*BC + j)
        full = np.stack(
            [np.asarray(results[c]["out"]).astype(np.float32).reshape(VPAD, T * BC) for c in range(NCORES)],
            axis=0,
        )
        full = full.reshape(NCORES, VPAD, T, BC)
        full = full.transpose(0, 3, 2, 1).reshape(BP, T, VPAD)
        logits = full[:B, :, :V].reshape(B * T, V)
        return np.ascontiguousarray(logits.astype(np.float32))

    return nc, in_maps, assemble


def kernel(**inputs):
    global LAST_RESULTS
    nc, in_maps, assemble = build_and_prep(inputs)

    from concourse.bass_utils import run_bass_kernel_spmd

    trace = bool(int(os.environ.get("KERNEL_TRACE", "0")))
    res = run_bass_kernel_spmd(
        nc, in_maps, core_ids=list(range(NCORES)), trace=trace
    )
    LAST_RESULTS = res
    return assemble(res.results)
